# revision 35
# baseline (speedup 1.0000x reference)
"""Trainium2 Bass kernel for nn_NNModel2 (2x NNConv GNN + pooled MLP readout).

Self-contained: accepts FULL inputs, shards edges across 8 NeuronCores
(edge-parallel by dst owner), returns the FULL [256, 1] output.

v2 design:
  - All gathers/transposes/broadcasts of *input-derived* data are done on the
    HOST and fed as per-core tensors (bf16): xsrcT, bcp (pair-broadcast attr),
    scatter one-hot matrices, permuted edge-MLP weights.
  - conv layer z-trick: z[e,(k,i)] = attr[e,k]*x[src,i]; msg = z @ W' done as
    PSUM-accumulated matmuls over 128-row (k,i) blocks. attr broadcast uses
    PAIR tiles (k0 on partitions 0:64, k1 on 64:128); conv2 covers full i-range
    with a partition-rotated copy of h1srcT (s=1 blocks).
  - h1 exchange via AllToAll of per-edge-needed rows (deduped per (src-owner,
    dst-owner) pair) instead of AllGather: ~0.7MB vs 2MB collective payload.
  - Tail: z1 partials computed locally, ReduceScatter over graphs, local
    readout of 32 graphs/core, AllGather of [256,1] result.
"""

import sys

sys.path.insert(0, "/opt/trn_rl_repo")

import numpy as np
import ml_dtypes

from concourse import bacc, bass, mybir
import concourse.tile as tile
from concourse import bass_utils

P = 128
NCORES = 8
N_NODES = 4096
N_EDGES = 8192
N_GRAPHS = 256
DN = 64
DE = 32
H = 256
NSH = N_NODES // NCORES  # 512
NT = NSH // P  # 4
GT = N_GRAPHS // P  # 2

F32 = mybir.dt.float32
BF16 = mybir.dt.bfloat16
I16 = mybir.dt.int16
AF = mybir.ActivationFunctionType
ALU = mybir.AluOpType
BF = ml_dtypes.bfloat16

_cache = {}
_PREP = {}


def _wrap_idx(idx, n):
    idx = np.asarray(idx, dtype=np.int16)
    assert idx.shape == (n,) and n % 16 == 0
    return np.tile(idx.reshape(n // 16, 16).T, (8, 1)).copy()


def _build(e_pad, S, sc_blocks, zb=(False, False, False), upto="full"):
    ET = e_pad // P
    SBT = S // P  # send-buffer tiles
    nc = bacc.Bacc(num_devices=NCORES)

    # ---- per-core inputs (host-prepped)
    xsrc2 = nc.dram_tensor("xsrc2", [P, 2, e_pad], BF16, kind="ExternalInput")
    bcq = nc.dram_tensor("bcq", [P, 8, e_pad], BF16, kind="ExternalInput")
    scm = nc.dram_tensor("scm", [P, len(sc_blocks) * P], BF16, kind="ExternalInput")
    scp = nc.dram_tensor("scp", [P, NT * GT * P], BF16, kind="ExternalInput")
    sel = nc.dram_tensor("sel", [P, (S // P) * NT * P], BF16, kind="ExternalInput")
    xshT = nc.dram_tensor("xshT", [DN + 1, NSH], BF16, kind="ExternalInput")
    h1src_w = nc.dram_tensor("h1src_w", [P, e_pad // 16], I16, kind="ExternalInput")
    identb = nc.dram_tensor("identb", [P, P], BF16, kind="ExternalInput")
    # ---- shared weights (host-permuted, bf16)
    w1p = nc.dram_tensor("w1p", [P, 16, H], BF16, kind="ExternalInput")
    w2p = nc.dram_tensor("w2p", [P, 64, H], BF16, kind="ExternalInput")
    b1p = nc.dram_tensor("b1p", [DN, H], BF16, kind="ExternalInput")
    b2p = nc.dram_tensor("b2p", [P, 2, H], BF16, kind="ExternalInput")
    r1wb = nc.dram_tensor("r1wb", [DN + 1, H], BF16, kind="ExternalInput")
    r2wb = nc.dram_tensor("r2wb", [P, 2, H], BF16, kind="ExternalInput")
    b2sbb = nc.dram_tensor("b2sbb", [1, H], BF16, kind="ExternalInput")
    l1wb = nc.dram_tensor("l1wb", [P, 2, H // 2], BF16, kind="ExternalInput")
    l1brow = nc.dram_tensor("l1brow", [1, H // 2], BF16, kind="ExternalInput")
    l2wrep = nc.dram_tensor("l2wrep", [N_GRAPHS // NCORES, H // 2], F32, kind="ExternalInput")
    l2brep = nc.dram_tensor("l2brep", [N_GRAPHS // NCORES, 1], F32, kind="ExternalInput")
    out = nc.dram_tensor("out", [N_GRAPHS, 1], F32, kind="ExternalOutput")

    def dbg_out(name, shape):
        return nc.dram_tensor(name, shape, F32, kind="ExternalOutput")

    zb1, zb2, zl1 = zb
    rg = [list(range(NCORES))]
    NSC = len(sc_blocks)
    GSH = N_GRAPHS // NCORES  # 32 graphs per core in the tail

    # first bank-touch bookkeeping for agg scatter (bank = n // 2)
    first_touch = {}
    for bi, (e, n) in enumerate(sc_blocks):
        first_touch.setdefault(n // 2, ("sc", bi))
    for n in range(NT):
        first_touch.setdefault(n // 2, ("root", n))

    with tile.TileContext(nc, num_cores=NCORES) as tc:
        with (
            tc.tile_pool(name="const", bufs=1) as cp,
            tc.tile_pool(name="work", bufs=3) as wp,
            tc.tile_pool(name="dram", bufs=1, space="DRAM") as dr,
        ):
            # ======== stage A: loads (SP queue), conv1-critical first.
            # Same-queue DMA transfers start in issue order, so priority ==
            # issue order here.
            bcq_sb = cp.tile([P, 8, e_pad], BF16)
            nc.sync.dma_start(out=bcq_sb[:, 0:2, :], in_=bcq[:, 0:2, :])
            xsrc2_sb = cp.tile([P, 2, e_pad], BF16)
            nc.sync.dma_start(out=xsrc2_sb[:], in_=xsrc2[:])
            w1p_sb = cp.tile([P, 16, H], BF16)
            nc.sync.dma_start(out=w1p_sb[:, 0:4, :], in_=w1p[:, 0:4, :])
            b1p_sb = cp.tile([DN, H], BF16)
            nc.sync.dma_start(out=b1p_sb[:], in_=b1p[:])
            for c in range(1, 4):
                nc.sync.dma_start(
                    out=bcq_sb[:, 2 * c : 2 * c + 2, :], in_=bcq[:, 2 * c : 2 * c + 2, :]
                )
                if c == 1:
                    nc.sync.dma_start(out=w1p_sb[:, 4:8, :], in_=w1p[:, 4:8, :])
                if c == 2:
                    nc.sync.dma_start(out=w1p_sb[:, 8:16, :], in_=w1p[:, 8:16, :])
            scm_sb = cp.tile([P, NSC * P], BF16)
            nc.sync.dma_start(out=scm_sb[:], in_=scm[:])
            xshT_sb = cp.tile([DN + 1, NSH], BF16)
            nc.sync.dma_start(out=xshT_sb[:], in_=xshT[:])
            r1wb_sb = cp.tile([DN + 1, H], BF16)
            nc.sync.dma_start(out=r1wb_sb[:], in_=r1wb[:])
            sel_sb = cp.tile([P, (S // P) * NT * P], BF16)
            nc.sync.dma_start(out=sel_sb[:], in_=sel[:])
            h1src_sb = cp.tile([P, e_pad // 16], I16)
            nc.sync.dma_start(out=h1src_sb[:], in_=h1src_w[:])
            ident_sb = cp.tile([P, P], BF16)
            nc.sync.dma_start(out=ident_sb[:], in_=identb[:])
            # conv2/tail loads last (small ones first, then the big w2p)
            a2a_in = dr.tile([S, H], BF16)
            b2p_sb = cp.tile([P, 2, H], BF16)
            nc.sync.dma_start(out=b2p_sb[:], in_=b2p[:])
            r2wb_sb = cp.tile([P, 2, H], BF16)
            nc.sync.dma_start(out=r2wb_sb[:], in_=r2wb[:])
            b2sbb_sb = cp.tile([1, H], BF16)
            nc.sync.dma_start(out=b2sbb_sb[:], in_=b2sbb[:])
            scp_sb = cp.tile([P, NT * GT * P], BF16)
            nc.sync.dma_start(out=scp_sb[:], in_=scp[:])
            l1wb_sb = cp.tile([P, 2, H // 2], BF16)
            nc.sync.dma_start(out=l1wb_sb[:], in_=l1wb[:])
            l1brow_sb = cp.tile([1, H // 2], BF16)
            nc.sync.dma_start(out=l1brow_sb[:], in_=l1brow[:])
            l2w_sb = cp.tile([GSH, H // 2], F32)
            nc.sync.dma_start(out=l2w_sb[:], in_=l2wrep[:])
            l2b_sb = cp.tile([GSH, 1], F32)
            nc.sync.dma_start(out=l2b_sb[:], in_=l2brep[:])
            w2p_sb = cp.tile([P, 64, H], BF16)
            for c in range(4):
                nc.sync.dma_start(
                    out=w2p_sb[:, 16 * c : 16 * c + 16, :],
                    in_=w2p[:, 16 * c : 16 * c + 16, :],
                )

            with tc.tile_pool(name="psA", bufs=1, space="PSUM") as psA:
                # ======== conv1
                msg_ps = [
                    psA.tile([P, 2 * H], F32, space="PSUM", tag=f"msg{j}", name=f"msg1_{j}")
                    for j in range((ET + 1) // 2)
                ]

                def m1(e):
                    return msg_ps[e // 2][:, (e % 2) * H : (e % 2) * H + H]

                if not zb1:
                    for e in range(ET):
                        nc.tensor.matmul(
                            m1(e), lhsT=xsrc2_sb[0:DN, 0, P * e : P * (e + 1)],
                            rhs=b1p_sb[:], start=(e % 2 == 0), stop=False,
                            skip_group_check=True,
                        )
                for t in range(16):
                    q1, s1 = t // 2, t % 2
                    zt = wp.tile([P, e_pad], BF16, tag="zt", bufs=4)
                    nc.vector.tensor_tensor(
                        out=zt[:], in0=xsrc2_sb[:, s1, :], in1=bcq_sb[:, q1, :],
                        op=ALU.mult,
                    )
                    for e in range(ET):
                        nc.tensor.matmul(
                            m1(e), lhsT=zt[:, P * e : P * (e + 1)], rhs=w1p_sb[:, t, :],
                            start=(zb1 and t == 0 and e % 2 == 0), stop=(t == 15),
                            skip_group_check=True,
                        )

                agg_ps = [
                    psA.tile([P, 2 * H], F32, space="PSUM", tag=f"agg{j}", name=f"agg1_{j}")
                    for j in range(NT // 2)
                ]

                def a1(n):
                    return agg_ps[n // 2][:, (n % 2) * H : (n % 2) * H + H]

                msbs = []
                for j in range((ET + 1) // 2):
                    w = min(2 * H, (ET - 2 * j) * H)
                    msb = wp.tile([P, 2 * H], BF16, tag="msb")
                    nc.scalar.activation(out=msb[:, 0:w], in_=msg_ps[j][:, 0:w], func=AF.Copy)
                    msbs.append(msb)

                ones_sb = cp.tile([1, P], BF16)
                nc.vector.memset(ones_sb[:], 1.0)

                def scatter_root(aget, msbs_l, root_lhs, bias_rhs):
                    for bi, (e, n) in enumerate(sc_blocks):
                        nc.tensor.matmul(
                            aget(n), lhsT=scm_sb[:, P * bi : P * (bi + 1)],
                            rhs=msbs_l[e // 2][:, (e % 2) * H : (e % 2) * H + H],
                            start=(first_touch[n // 2] == ("sc", bi)), stop=False,
                            skip_group_check=True,
                        )
                    for n in range(NT):
                        pairs = root_lhs(n)
                        for li, (lhs, rhs) in enumerate(pairs):
                            last = bias_rhs is None and li == len(pairs) - 1
                            nc.tensor.matmul(
                                aget(n), lhsT=lhs, rhs=rhs,
                                start=(first_touch[n // 2] == ("root", n) and li == 0),
                                stop=last, skip_group_check=True,
                            )
                        if bias_rhs is not None:
                            nc.tensor.matmul(
                                aget(n), lhsT=ones_sb[:], rhs=bias_rhs,
                                start=False, stop=True, skip_group_check=True,
                            )

                def root1(n):
                    return [(xshT_sb[:, P * n : P * (n + 1)], r1wb_sb[:])]

                # bias1 is folded into r1wb (row 64 = ones in xshT)
                scatter_root(a1, msbs, root1, None)

                h1sb = cp.tile([P, NT, H], BF16)
                for j in range(NT // 2):
                    nc.scalar.activation(
                        out=h1sb[:, 2 * j : 2 * j + 2, :], in_=agg_ps[j][:, 0 : 2 * H],
                        func=AF.Relu,
                    )

                if upto == "h1":
                    dh = dbg_out("d_h1", [P, NT * H])
                    tmp = wp.tile([P, NT, H], F32, tag="dbgf")
                    nc.vector.tensor_copy(out=tmp[:], in_=h1sb[:])
                    nc.sync.dma_start(
                        out=dh[:].rearrange("p (t o) -> p t o", o=H), in_=tmp[:]
                    )

                # ======== exchange: sendbuf rows via one-hot matmuls -> AllToAll
                snd_ps = [
                    psA.tile([P, 2 * H], F32, space="PSUM", tag=f"msg{j}", name=f"snd_{j}")
                    for j in range((SBT + 1) // 2)
                ]

                def sb_ps(r):
                    return snd_ps[r // 2][:, (r % 2) * H : (r % 2) * H + H]

                for r in range(SBT):
                    for n in range(NT):
                        blk = r * NT + n
                        nc.tensor.matmul(
                            sb_ps(r), lhsT=sel_sb[:, P * blk : P * (blk + 1)],
                            rhs=h1sb[:, n, :], start=(n == 0 and r % 2 == 0),
                            stop=(n == NT - 1), skip_group_check=True,
                        )
                sendbuf = cp.tile([P, 2 * ((SBT + 1) // 2), H], BF16)
                for j in range((SBT + 1) // 2):
                    if (SBT - 2 * j) >= 2:
                        nc.scalar.activation(
                            out=sendbuf[:, 2 * j : 2 * j + 2, :],
                            in_=snd_ps[j][:, 0 : 2 * H], func=AF.Copy,
                        )
                    else:
                        nc.scalar.activation(
                            out=sendbuf[:, 2 * j, :], in_=snd_ps[j][:, 0:H], func=AF.Copy,
                        )
                nc.gpsimd.dma_start(
                    out=a2a_in[:].rearrange("(b p) e -> p b e", p=P),
                    in_=sendbuf[:, 0:SBT, :],
                )
                a2a_out = dr.tile([S, H], BF16)
                nc.gpsimd.collective_compute(
                    "AllToAll", ALU.bypass, replica_groups=rg,
                    ins=[a2a_in[:].opt()], outs=[a2a_out[:].opt()],
                )
                h1srcT = cp.tile([P, 2, e_pad], BF16)
                nc.gpsimd.dma_gather(
                    out_ap=h1srcT[:], in_ap=a2a_out[:], idxs_ap=h1src_sb[:],
                    num_idxs=e_pad, num_idxs_reg=e_pad, elem_size=H,
                    transpose=True, single_packet=False,
                )
                # h1shT via PE transposes of h1sb (PE is idle during the
                # AllToAll; alternating psum tags pipeline transpose+copy)
                h1shT = cp.tile([P, 2, NSH], BF16)
                for n in range(NT):
                    for oh in range(2):
                        tsh = psA.tile(
                            [P, P], BF16, space="PSUM", tag=f"agg{(n * 2 + oh) % 2}",
                            name=f"tsh_{n}_{oh}",
                        )
                        nc.tensor.transpose(
                            out=tsh[:], in_=h1sb[:, n, P * oh : P * (oh + 1)],
                            identity=ident_sb[:],
                        )
                        nc.scalar.activation(
                            out=h1shT[:, oh, P * n : P * (n + 1)], in_=tsh[:],
                            func=AF.Copy,
                        )
                # rotated copies for s=1..3: h1rot_r[p,c] = feat[(128c+p+32r)%256]
                # (32-partition aligned chunks -- walrus rejects unaligned
                # partition-offset spans)
                h1rots = [h1srcT]
                for r in range(1, 4):
                    h1r = cp.tile([P, 2, e_pad], BF16, name=f"h1rot{r}")
                    for c in range(2):
                        for d in range(4):
                            t = 32 * (d + r)
                            q, slot = t % P, (c if t < P else 1 - c)
                            nc.vector.tensor_copy(
                                out=h1r[32 * d : 32 * d + 32, c, :],
                                in_=h1srcT[q : q + 32, slot, :],
                            )
                    h1rots.append(h1r)

                if upto == "h1srcT":
                    d1 = dbg_out("d_h1srcT", [P, 2 * e_pad])
                    tmp = wp.tile([P, 2, e_pad], F32, tag="dbgf")
                    nc.vector.tensor_copy(out=tmp[:], in_=h1srcT[:])
                    nc.sync.dma_start(
                        out=d1[:].rearrange("p (c e) -> p c e", c=2), in_=tmp[:]
                    )

                # ======== conv2: 64 blocks, s-major (s=0 first)
                msg2_ps = [
                    psA.tile([P, 2 * H], F32, space="PSUM", tag=f"msg{j}", name=f"msg2_{j}")
                    for j in range((ET + 1) // 2)
                ]

                def m2(e):
                    return msg2_ps[e // 2][:, (e % 2) * H : (e % 2) * H + H]

                if not zb2:
                    for e in range(ET):
                        for ih in range(2):
                            nc.tensor.matmul(
                                m2(e), lhsT=h1srcT[:, ih, P * e : P * (e + 1)],
                                rhs=b2p_sb[:, ih, :], start=(ih == 0 and e % 2 == 0),
                                stop=False, skip_group_check=True,
                            )
                for b in range(64):
                    s2, q2, ih = b // 16, (b % 16) // 2, b % 2
                    srct = h1rots[s2]
                    zt = wp.tile([P, e_pad], BF16, tag="zt", bufs=4)
                    nc.vector.tensor_tensor(
                        out=zt[:], in0=srct[:, ih, :], in1=bcq_sb[:, q2, :], op=ALU.mult
                    )
                    for e in range(ET):
                        nc.tensor.matmul(
                            m2(e), lhsT=zt[:, P * e : P * (e + 1)], rhs=w2p_sb[:, b, :],
                            start=(zb2 and b == 0 and e % 2 == 0), stop=(b == 63),
                            skip_group_check=True,
                        )

                agg2_ps = [
                    psA.tile([P, 2 * H], F32, space="PSUM", tag=f"agg{j}", name=f"agg2_{j}")
                    for j in range(NT // 2)
                ]

                def a2(n):
                    return agg2_ps[n // 2][:, (n % 2) * H : (n % 2) * H + H]

                msbs2 = []
                for j in range((ET + 1) // 2):
                    w = min(2 * H, (ET - 2 * j) * H)
                    msb = wp.tile([P, 2 * H], BF16, tag="msb")
                    nc.scalar.activation(out=msb[:, 0:w], in_=msg2_ps[j][:, 0:w], func=AF.Copy)
                    msbs2.append(msb)

                def root2(n):
                    return [
                        (h1shT[:, kh, P * n : P * (n + 1)], r2wb_sb[:, kh, :])
                        for kh in range(2)
                    ]

                scatter_root(a2, msbs2, root2, None if zb2 else b2sbb_sb[:])

                h2sb = cp.tile([P, NT, H], BF16)
                for j in range(NT // 2):
                    nc.scalar.activation(
                        out=h2sb[:, 2 * j : 2 * j + 2, :], in_=agg2_ps[j][:, 0 : 2 * H],
                        func=AF.Copy,
                    )

                if upto == "h2":
                    dh = dbg_out("d_h2", [P, NT * H])
                    tmp = wp.tile([P, NT, H], F32, tag="dbgf")
                    nc.vector.tensor_copy(out=tmp[:], in_=h2sb[:])
                    nc.sync.dma_start(
                        out=dh[:].rearrange("p (t o) -> p t o", o=H), in_=tmp[:]
                    )

                # ======== pool (transposed, recip folded into scp) + z1T partials
                # meanT_ps[:, oh, g*128:...] = sum_n h2sb[:,n,128oh:].T @ scp_blk(n,g)
                meanT_ps = psA.tile([P, 2, H], F32, space="PSUM", tag="agg0", name="meanT")
                for n in range(NT):
                    for oh in range(2):
                        for g in range(GT):
                            blk = n * GT + g
                            nc.tensor.matmul(
                                meanT_ps[:, oh, P * g : P * (g + 1)],
                                lhsT=h2sb[:, n, P * oh : P * (oh + 1)],
                                rhs=scp_sb[:, P * blk : P * (blk + 1)],
                                start=(n == 0 and oh == 0 and g == 0),
                                stop=(n == NT - 1 and oh == 1 and g == GT - 1),
                                skip_group_check=True,
                            )
                meanT_sb = cp.tile([P, 2, H], BF16)
                nc.scalar.activation(out=meanT_sb[:], in_=meanT_ps[:], func=AF.Copy)
                # z1T[g, m] = sum_h meanT[h, g] * l1w[h, m]  (+ l1b/8 via ones row)
                z1T_ps = psA.tile([P, GT, H // 2], F32, space="PSUM", tag="agg1", name="z1T")
                for g in range(GT):
                    for oh in range(2):
                        nc.tensor.matmul(
                            z1T_ps[:, g, :],
                            lhsT=meanT_sb[:, oh, P * g : P * (g + 1)],
                            rhs=l1wb_sb[:, oh, :],
                            start=(g == 0 and oh == 0),
                            stop=(zl1 and g == GT - 1 and oh == 1),
                            skip_group_check=True,
                        )
                    if not zl1:
                        nc.tensor.matmul(
                            z1T_ps[:, g, :], lhsT=ones_sb[:], rhs=l1brow_sb[:],
                            start=False, stop=(g == GT - 1), skip_group_check=True,
                        )
                z1T = cp.tile([P, GT, H // 2], F32)
                nc.vector.tensor_copy(out=z1T[:], in_=z1T_ps[:])
                rs_in = dr.tile([N_GRAPHS, H // 2], F32)
                nc.sync.dma_start(
                    out=rs_in[:].rearrange("(g p) m -> p g m", p=P), in_=z1T[:]
                )

            # ======== tail: ReduceScatter, local readout, AllGather
            with tc.tile_pool(name="psB", bufs=1, space="PSUM") as psB:
                rs_out = dr.tile([GSH, H // 2], F32)
                nc.gpsimd.collective_compute(
                    "ReduceScatter", ALU.add, replica_groups=rg,
                    ins=[rs_in[:].opt()], outs=[rs_out[:].opt()],
                )
                # ======== local readout of GSH graphs
                rs_sb = cp.tile([GSH, H // 2], F32)
                nc.sync.dma_start(out=rs_sb[:], in_=rs_out[:])
                # fused relu(x) * l2w with free-dim reduction in one DVE op
                prod = wp.tile([GSH, H // 2], F32, tag="t2")
                red = wp.tile([GSH, 1], F32, tag="t3")
                nc.vector.scalar_tensor_tensor(
                    out=prod[:], in0=rs_sb[:], scalar=0.0, in1=l2w_sb[:],
                    op0=ALU.max, op1=ALU.mult, accum_out=red[:],
                )
                osb = wp.tile([GSH, 1], F32, tag="t4")
                nc.scalar.activation(
                    out=osb[:], in_=red[:], func=AF.Sigmoid, bias=l2b_sb[:, 0:1]
                )
                ag_in = dr.tile([GSH, 1], F32)
                nc.sync.dma_start(out=ag_in[:], in_=osb[:])
                ag_out = dr.tile([N_GRAPHS, 1], F32, addr_space="Shared")
                nc.gpsimd.collective_compute(
                    "AllGather", ALU.bypass, replica_groups=rg,
                    ins=[ag_in[:].opt()], outs=[ag_out[:].opt()],
                )
                nc.sync.dma_start(out=out[:], in_=ag_out[:])

    nc.compile()
    return nc


def _prep_inputs(inputs):
    x = np.asarray(inputs["x"], dtype=np.float32)
    ei = np.asarray(inputs["edge_index"])
    attr = np.asarray(inputs["edge_attr"], dtype=np.float32)
    batch = np.asarray(inputs["batch"]).astype(np.int64)
    src, dst = ei[0].astype(np.int64), ei[1].astype(np.int64)

    owner = dst // NSH
    per_core = []
    for c in range(NCORES):
        eids = np.nonzero(owner == c)[0]
        eids = eids[np.argsort(dst[eids], kind="stable")]
        per_core.append(eids)
    need = max(max(len(e) for e in per_core), 1)
    e_pad = max(((need + P - 1) // P) * P, P)
    ET = e_pad // P

    # static union of scatter blocks (e_tile, n_tile)
    blocks = set()
    for c in range(NCORES):
        dstl = dst[per_core[c]] - c * NSH
        for e in range(ET):
            seg = dstl[e * P : (e + 1) * P]
            if len(seg) == 0:
                continue
            for n in range(int(seg.min()) // P, int(seg.max()) // P + 1):
                blocks.add((e, int(n)))
    sc_blocks = sorted(blocks)
    NSC = len(sc_blocks)

    # A2A send rows (dedup per (sender c, receiver d) pair) and receive mapping
    send_rows = [[None] * NCORES for _ in range(NCORES)]
    recv_pos_parts = [[None] * NCORES for _ in range(NCORES)]  # [d][c]
    maxrows = 1
    for d in range(NCORES):
        eids = per_core[d]
        srcs = src[eids]
        co = srcs // NSH
        for c in range(NCORES):
            mask = co == c
            uniq, inv = np.unique(srcs[mask] - c * NSH, return_inverse=True)
            send_rows[c][d] = uniq
            recv_pos_parts[d][c] = (np.nonzero(mask)[0], inv)
            maxrows = max(maxrows, len(uniq))
    SB = ((maxrows + 15) // 16) * 16
    S = NCORES * SB

    # host-permuted weights (shared)
    nn1_w = np.asarray(inputs["nn1_w"], np.float32)  # [32, 64*256]
    nn2_w = np.asarray(inputs["nn2_w"], np.float32)  # [32, 256*256]
    pidx = np.arange(P)
    g32 = pidx // 32
    j32 = pidx % 32
    nn1_r = nn1_w.reshape(DE, DN, H)
    w1p = np.zeros((P, 16, H), np.float32)
    for t in range(16):
        q, s = t // 2, t % 2
        k = 4 * q + g32
        i = (32 * (g32 + s) + j32) % DN
        w1p[:, t, :] = nn1_r[k, i, :]
    w1p = w1p.astype(BF)
    nn2_r = nn2_w.reshape(DE, H, H)
    w2p = np.zeros((P, 64, H), np.float32)
    for b in range(64):
        s, q, ih = b // 16, (b % 16) // 2, b % 2
        k = 4 * q + g32
        i = (128 * ih + 32 * (g32 + s) + j32) % H
        w2p[:, b, :] = nn2_r[k, i, :]
    w2p = w2p.astype(BF)

    nn1_b = np.asarray(inputs["nn1_b"], np.float32).reshape(DN, H)
    nn2_b = np.asarray(inputs["nn2_b"], np.float32).reshape(H, H)
    b2p = np.stack([nn2_b[0:P, :], nn2_b[P : 2 * P, :]], axis=1)  # [128, 2, 256]
    r1w = np.asarray(inputs["root1_w"], np.float32)
    bias1 = np.asarray(inputs["bias1"], np.float32)
    r1wb = np.concatenate([r1w, bias1.reshape(1, H)], axis=0)  # [65, 256]
    r2w = np.asarray(inputs["root2_w"], np.float32)
    r2wb = np.stack([r2w[0:P, :], r2w[P : 2 * P, :]], axis=1)  # [128, 2, 256]
    bias2 = np.asarray(inputs["bias2"], np.float32).reshape(1, H)
    l1w = np.asarray(inputs["lin1_w"], np.float32)  # [256, 128]
    l1wb = np.stack([l1w[0:P, :], l1w[P : 2 * P, :]], axis=1)  # [128, 2, 128]
    l1b = np.asarray(inputs["lin1_b"], np.float32).reshape(1, H // 2)
    l2w = np.asarray(inputs["lin2_w"], np.float32).reshape(1, H // 2)
    l2b = np.asarray(inputs["lin2_b"], np.float32).reshape(1, 1)
    GSH = N_GRAPHS // NCORES

    cnt = np.bincount(batch, minlength=N_GRAPHS).astype(np.float32)
    recip_g = 1.0 / np.maximum(cnt, 1.0)  # [256], per graph

    common = {
        "w1p": w1p, "w2p": w2p,
        "b1p": nn1_b.astype(BF), "b2p": b2p.astype(BF),
        "r1wb": r1wb.astype(BF), "r2wb": r2wb.astype(BF),
        "b2sbb": bias2.astype(BF),
        "l1wb": l1wb.astype(BF), "l1brow": (l1b / NCORES).astype(BF),
        "l2wrep": np.tile(l2w, (GSH, 1)).astype(np.float32),
        "l2brep": np.tile(l2b, (GSH, 1)).astype(np.float32),
        "identb": np.eye(P, dtype=BF),
    }

    in_maps = []
    for c in range(NCORES):
        eids = per_core[c]
        ne = len(eids)
        srcs = src[eids]
        dstl = (dst[eids] - c * NSH).astype(np.int64)

        xg = x[srcs, :].astype(BF)  # [ne, 64]
        xsrc2 = np.zeros((P, 2, e_pad), BF)
        for s in range(2):
            iofs = (32 * (g32 + s) + j32) % DN  # [128]
            xsrc2[:, s, 0:ne] = xg[:, iofs].T

        ag = attr[eids, :]  # [ne, 32]
        bcq = np.zeros((P, 8, e_pad), BF)
        for q in range(8):
            for g in range(4):
                bcq[32 * g : 32 * g + 32, q, 0:ne] = ag[:, 4 * q + g].astype(BF)[None, :]

        scm = np.zeros((P, NSC * P), BF)
        for bi, (e, n) in enumerate(sc_blocks):
            seg = dstl[e * P : min((e + 1) * P, ne)]
            for p, dv in enumerate(seg):
                q = dv - n * P
                if 0 <= q < P:
                    scm[p, bi * P + q] = 1.0

        batch_l = batch[c * NSH : (c + 1) * NSH]
        scp = np.zeros((P, NT * GT * P), BF)
        for n in range(NT):
            for g in range(GT):
                blk = n * GT + g
                bseg = batch_l[n * P : (n + 1) * P]
                for p, bv in enumerate(bseg):
                    q = bv - g * P
                    if 0 <= q < P:
                        scp[p, blk * P + q] = BF(recip_g[bv])

        xshT = np.ones((DN + 1, NSH), BF)
        xshT[0:DN, :] = x[c * NSH : (c + 1) * NSH, :].astype(BF).T

        snd_idx = np.full(S, -1, np.int64)
        for d in range(NCORES):
            rows = send_rows[c][d]
            snd_idx[d * SB : d * SB + len(rows)] = rows
        SBT = S // P
        selm = np.zeros((P, SBT * NT * P), BF)
        for row in range(S):
            v = snd_idx[row]
            if v < 0:
                continue
            r, q = row // P, row % P
            nt_, npart = int(v) // P, int(v) % P
            selm[npart, (r * NT + nt_) * P + q] = 1.0
        h1src_idx = np.zeros(e_pad, np.int16)
        for d2 in range(NCORES):
            pos, inv = recv_pos_parts[c][d2]
            h1src_idx[pos] = d2 * SB + inv

        m = dict(common)
        m["xsrc2"] = xsrc2
        m["bcq"] = bcq
        m["scm"] = scm
        m["scp"] = scp
        m["sel"] = selm
        m["xshT"] = xshT
        m["h1src_w"] = _wrap_idx(h1src_idx, e_pad)
        in_maps.append(m)

    zb = (
        bool(np.all(np.asarray(inputs["nn1_b"]) == 0)),
        bool(np.all(np.asarray(inputs["nn2_b"]) == 0))
        and bool(np.all(np.asarray(inputs["bias2"]) == 0)),
        bool(np.all(np.asarray(inputs["lin1_b"]) == 0)),
    )
    _PREP["args"] = (e_pad, S, tuple(sc_blocks), zb)
    return e_pad, in_maps


def kernel(**inputs) -> np.ndarray:
    e_pad, in_maps = _prep_inputs(inputs)
    if e_pad not in _cache:
        ep, S, blocks, zb = _PREP["args"]
        _cache[e_pad] = _build(ep, S, list(blocks), zb=zb)
    nc = _cache[e_pad]
    res = bass_utils.run_bass_kernel_spmd(nc, in_maps, core_ids=list(range(NCORES)))
    return np.asarray(res.results[0]["out"], dtype=np.float32)


def run_debug(upto, **inputs):
    e_pad, in_maps = _prep_inputs(inputs)
    ep, S, blocks, zb = _PREP["args"]
    nc = _build(ep, S, list(blocks), zb=zb, upto=upto)
    res = bass_utils.run_bass_kernel_spmd(nc, in_maps, core_ids=list(range(NCORES)))
    return e_pad, res


# revision 38
# speedup vs baseline: 1.0566x; 1.0566x over previous
"""Trainium2 Bass kernel for nn_NNModel2 (2x NNConv GNN + pooled MLP readout).

Self-contained: accepts FULL inputs, shards edges across 8 NeuronCores
(edge-parallel by dst owner), returns the FULL [256, 1] output.

v2 design:
  - All gathers/transposes/broadcasts of *input-derived* data are done on the
    HOST and fed as per-core tensors (bf16): xsrcT, bcp (pair-broadcast attr),
    scatter one-hot matrices, permuted edge-MLP weights.
  - conv layer z-trick: z[e,(k,i)] = attr[e,k]*x[src,i]; msg = z @ W' done as
    PSUM-accumulated matmuls over 128-row (k,i) blocks. attr broadcast uses
    PAIR tiles (k0 on partitions 0:64, k1 on 64:128); conv2 covers full i-range
    with a partition-rotated copy of h1srcT (s=1 blocks).
  - h1 exchange via AllToAll of per-edge-needed rows (deduped per (src-owner,
    dst-owner) pair) instead of AllGather: ~0.7MB vs 2MB collective payload.
  - Tail: z1 partials computed locally, ReduceScatter over graphs, local
    readout of 32 graphs/core, AllGather of [256,1] result.
"""

import sys

sys.path.insert(0, "/opt/trn_rl_repo")

import numpy as np
import ml_dtypes

from concourse import bacc, bass, mybir
import concourse.tile as tile
from concourse import bass_utils

P = 128
NCORES = 8
N_NODES = 4096
N_EDGES = 8192
N_GRAPHS = 256
DN = 64
DE = 32
H = 256
NSH = N_NODES // NCORES  # 512
NT = NSH // P  # 4
GT = N_GRAPHS // P  # 2

F32 = mybir.dt.float32
BF16 = mybir.dt.bfloat16
I16 = mybir.dt.int16
AF = mybir.ActivationFunctionType
ALU = mybir.AluOpType
BF = ml_dtypes.bfloat16

_cache = {}
_PREP = {}


def _wrap_idx(idx, n):
    idx = np.asarray(idx, dtype=np.int16)
    assert idx.shape == (n,) and n % 16 == 0
    return np.tile(idx.reshape(n // 16, 16).T, (8, 1)).copy()


def _build(e_pad, S, sc_blocks, zb=(False, False, False), upto="full"):
    ET = e_pad // P
    SBT = S // P  # send-buffer tiles
    nc = bacc.Bacc(num_devices=NCORES)

    # ---- per-core inputs (host-prepped)
    xsrc2 = nc.dram_tensor("xsrc2", [P, 2, e_pad], BF16, kind="ExternalInput")
    bcq = nc.dram_tensor("bcq", [P, 8, e_pad], BF16, kind="ExternalInput")
    scm = nc.dram_tensor("scm", [P, len(sc_blocks) * P], BF16, kind="ExternalInput")
    scp = nc.dram_tensor("scp", [P, NT * GT * P], BF16, kind="ExternalInput")
    sel = nc.dram_tensor("sel", [P, (S // P) * NT * P], BF16, kind="ExternalInput")
    xshT = nc.dram_tensor("xshT", [DN + 1, NSH], BF16, kind="ExternalInput")
    h1src_w = nc.dram_tensor("h1src_w", [P, e_pad // 16], I16, kind="ExternalInput")
    identb = nc.dram_tensor("identb", [P, P], BF16, kind="ExternalInput")
    # ---- shared weights (host-permuted, bf16)
    w1p = nc.dram_tensor("w1p", [P, 16, H], BF16, kind="ExternalInput")
    w2p = nc.dram_tensor("w2p", [P, 64, H], BF16, kind="ExternalInput")
    b1p = nc.dram_tensor("b1p", [DN, H], BF16, kind="ExternalInput")
    b2p = nc.dram_tensor("b2p", [P, 2, H], BF16, kind="ExternalInput")
    r1wb = nc.dram_tensor("r1wb", [DN + 1, H], BF16, kind="ExternalInput")
    r2wb = nc.dram_tensor("r2wb", [P, 2, H], BF16, kind="ExternalInput")
    b2sbb = nc.dram_tensor("b2sbb", [1, H], BF16, kind="ExternalInput")
    l1wb = nc.dram_tensor("l1wb", [P, 2, H // 2], BF16, kind="ExternalInput")
    l1brow = nc.dram_tensor("l1brow", [1, H // 2], BF16, kind="ExternalInput")
    l2wrep = nc.dram_tensor("l2wrep", [N_GRAPHS // NCORES, H // 2], F32, kind="ExternalInput")
    l2brep = nc.dram_tensor("l2brep", [N_GRAPHS // NCORES, 1], F32, kind="ExternalInput")
    out = nc.dram_tensor("out", [N_GRAPHS, 1], F32, kind="ExternalOutput")

    def dbg_out(name, shape):
        return nc.dram_tensor(name, shape, F32, kind="ExternalOutput")

    zb1, zb2, zl1 = zb
    rg = [list(range(NCORES))]
    NSC = len(sc_blocks)
    GSH = N_GRAPHS // NCORES  # 32 graphs per core in the tail

    # first bank-touch bookkeeping for agg scatter (bank = n // 2)
    first_touch = {}
    for bi, (e, n) in enumerate(sc_blocks):
        first_touch.setdefault(n // 2, ("sc", bi))
    for n in range(NT):
        first_touch.setdefault(n // 2, ("root", n))

    with tile.TileContext(nc, num_cores=NCORES) as tc:
        with (
            tc.tile_pool(name="const", bufs=1) as cp,
            tc.tile_pool(name="work", bufs=3) as wp,
            tc.tile_pool(name="dram", bufs=1, space="DRAM") as dr,
        ):
            # ======== stage A: loads (SP queue), conv1-critical first.
            # Same-queue DMA transfers start in issue order, so priority ==
            # issue order here.
            bcq_sb = cp.tile([P, 8, e_pad], BF16)
            nc.sync.dma_start(out=bcq_sb[:, 0:2, :], in_=bcq[:, 0:2, :])
            xsrc2_sb = cp.tile([P, 2, e_pad], BF16)
            nc.sync.dma_start(out=xsrc2_sb[:], in_=xsrc2[:])
            w1p_sb = cp.tile([P, 16, H], BF16)
            nc.sync.dma_start(out=w1p_sb[:, 0:4, :], in_=w1p[:, 0:4, :])
            b1p_sb = cp.tile([DN, H], BF16)
            nc.sync.dma_start(out=b1p_sb[:], in_=b1p[:])
            for c in range(1, 4):
                nc.sync.dma_start(
                    out=bcq_sb[:, 2 * c : 2 * c + 2, :], in_=bcq[:, 2 * c : 2 * c + 2, :]
                )
                if c == 1:
                    nc.sync.dma_start(out=w1p_sb[:, 4:8, :], in_=w1p[:, 4:8, :])
                if c == 2:
                    nc.sync.dma_start(out=w1p_sb[:, 8:16, :], in_=w1p[:, 8:16, :])
            scm_sb = cp.tile([P, NSC * P], BF16)
            nc.sync.dma_start(out=scm_sb[:], in_=scm[:])
            xshT_sb = cp.tile([DN + 1, NSH], BF16)
            nc.sync.dma_start(out=xshT_sb[:], in_=xshT[:])
            r1wb_sb = cp.tile([DN + 1, H], BF16)
            nc.sync.dma_start(out=r1wb_sb[:], in_=r1wb[:])
            sel_sb = cp.tile([P, (S // P) * NT * P], BF16)
            nc.sync.dma_start(out=sel_sb[:], in_=sel[:])
            h1src_sb = cp.tile([P, e_pad // 16], I16)
            nc.sync.dma_start(out=h1src_sb[:], in_=h1src_w[:])
            ident_sb = cp.tile([P, P], BF16)
            nc.sync.dma_start(out=ident_sb[:], in_=identb[:])
            # conv2/tail loads last (small ones first, then the big w2p)
            a2a_in = dr.tile([S, H], BF16)
            b2p_sb = cp.tile([P, 2, H], BF16)
            nc.sync.dma_start(out=b2p_sb[:], in_=b2p[:])
            r2wb_sb = cp.tile([P, 2, H], BF16)
            nc.sync.dma_start(out=r2wb_sb[:], in_=r2wb[:])
            b2sbb_sb = cp.tile([1, H], BF16)
            nc.sync.dma_start(out=b2sbb_sb[:], in_=b2sbb[:])
            scp_sb = cp.tile([P, NT * GT * P], BF16)
            nc.sync.dma_start(out=scp_sb[:], in_=scp[:])
            l1wb_sb = cp.tile([P, 2, H // 2], BF16)
            nc.sync.dma_start(out=l1wb_sb[:], in_=l1wb[:])
            l1brow_sb = cp.tile([1, H // 2], BF16)
            nc.sync.dma_start(out=l1brow_sb[:], in_=l1brow[:])
            l2w_sb = cp.tile([GSH, H // 2], F32)
            nc.sync.dma_start(out=l2w_sb[:], in_=l2wrep[:])
            l2b_sb = cp.tile([GSH, 1], F32)
            nc.sync.dma_start(out=l2b_sb[:], in_=l2brep[:])
            w2p_sb = cp.tile([P, 64, H], BF16)
            for c in range(4):
                nc.sync.dma_start(
                    out=w2p_sb[:, 16 * c : 16 * c + 16, :],
                    in_=w2p[:, 16 * c : 16 * c + 16, :],
                )

            with tc.tile_pool(name="psA", bufs=1, space="PSUM") as psA:
                # ======== conv1
                msg_ps = [
                    psA.tile([P, 2 * H], F32, space="PSUM", tag=f"msg{j}", name=f"msg1_{j}")
                    for j in range((ET + 1) // 2)
                ]

                def m1(e):
                    return msg_ps[e // 2][:, (e % 2) * H : (e % 2) * H + H]

                msbs = []

                zts1 = []
                for t in range(16):
                    q1, s1 = t // 2, t % 2
                    zt = wp.tile([P, e_pad], BF16, tag=f"zt1_{t}", name=f"zt1_{t}", bufs=1)
                    nc.vector.tensor_tensor(
                        out=zt[:], in0=xsrc2_sb[:, s1, :], in1=bcq_sb[:, q1, :],
                        op=ALU.mult,
                    )
                    zts1.append(zt)
                # e-major accumulation: each msg bank closes early so its
                # PSUM->SBUF copy overlaps the remaining matmuls
                for e in range(ET):
                    if not zb1:
                        nc.tensor.matmul(
                            m1(e), lhsT=xsrc2_sb[0:DN, 0, P * e : P * (e + 1)],
                            rhs=b1p_sb[:], start=(e % 2 == 0), stop=False,
                            skip_group_check=True,
                        )
                    for t in range(16):
                        nc.tensor.matmul(
                            m1(e), lhsT=zts1[t][:, P * e : P * (e + 1)],
                            rhs=w1p_sb[:, t, :],
                            start=(zb1 and t == 0 and e % 2 == 0), stop=(t == 15),
                            skip_group_check=True,
                        )
                    if e % 2 == 1 or e == ET - 1:
                        j = e // 2
                        w = min(2 * H, (ET - 2 * j) * H)
                        msb = wp.tile([P, 2 * H], BF16, tag="msb", bufs=5, name=f"msb1_{j}")
                        nc.scalar.activation(
                            out=msb[:, 0:w], in_=msg_ps[j][:, 0:w], func=AF.Copy
                        )
                        msbs.append(msb)

                agg_ps = [
                    psA.tile([P, 2 * H], F32, space="PSUM", tag=f"agg{j}", name=f"agg1_{j}")
                    for j in range(NT // 2)
                ]

                def a1(n):
                    return agg_ps[n // 2][:, (n % 2) * H : (n % 2) * H + H]


                ones_sb = cp.tile([1, P], BF16)
                nc.vector.memset(ones_sb[:], 1.0)

                def scatter_root(aget, msbs_l, root_lhs, bias_rhs):
                    for bi, (e, n) in enumerate(sc_blocks):
                        nc.tensor.matmul(
                            aget(n), lhsT=scm_sb[:, P * bi : P * (bi + 1)],
                            rhs=msbs_l[e // 2][:, (e % 2) * H : (e % 2) * H + H],
                            start=(first_touch[n // 2] == ("sc", bi)), stop=False,
                            skip_group_check=True,
                        )
                    for n in range(NT):
                        pairs = root_lhs(n)
                        for li, (lhs, rhs) in enumerate(pairs):
                            last = bias_rhs is None and li == len(pairs) - 1
                            nc.tensor.matmul(
                                aget(n), lhsT=lhs, rhs=rhs,
                                start=(first_touch[n // 2] == ("root", n) and li == 0),
                                stop=last, skip_group_check=True,
                            )
                        if bias_rhs is not None:
                            nc.tensor.matmul(
                                aget(n), lhsT=ones_sb[:], rhs=bias_rhs,
                                start=False, stop=True, skip_group_check=True,
                            )

                def root1(n):
                    return [(xshT_sb[:, P * n : P * (n + 1)], r1wb_sb[:])]

                # bias1 is folded into r1wb (row 64 = ones in xshT)
                scatter_root(a1, msbs, root1, None)

                h1sb = cp.tile([P, NT, H], BF16)
                for j in range(NT // 2):
                    nc.scalar.activation(
                        out=h1sb[:, 2 * j : 2 * j + 2, :], in_=agg_ps[j][:, 0 : 2 * H],
                        func=AF.Relu,
                    )

                if upto == "h1":
                    dh = dbg_out("d_h1", [P, NT * H])
                    tmp = wp.tile([P, NT, H], F32, tag="dbgf")
                    nc.vector.tensor_copy(out=tmp[:], in_=h1sb[:])
                    nc.sync.dma_start(
                        out=dh[:].rearrange("p (t o) -> p t o", o=H), in_=tmp[:]
                    )

                # ======== exchange: sendbuf rows via one-hot matmuls -> AllToAll
                snd_ps = [
                    psA.tile([P, 2 * H], F32, space="PSUM", tag=f"msg{j}", name=f"snd_{j}")
                    for j in range((SBT + 1) // 2)
                ]

                def sb_ps(r):
                    return snd_ps[r // 2][:, (r % 2) * H : (r % 2) * H + H]

                sendbuf = cp.tile([P, 2 * ((SBT + 1) // 2), H], BF16)
                for r in range(SBT):
                    for n in range(NT):
                        blk = r * NT + n
                        nc.tensor.matmul(
                            sb_ps(r), lhsT=sel_sb[:, P * blk : P * (blk + 1)],
                            rhs=h1sb[:, n, :], start=(n == 0 and r % 2 == 0),
                            stop=(n == NT - 1), skip_group_check=True,
                        )
                    if r % 2 == 1 or r == SBT - 1:
                        j = r // 2
                        if (SBT - 2 * j) >= 2:
                            nc.scalar.activation(
                                out=sendbuf[:, 2 * j : 2 * j + 2, :],
                                in_=snd_ps[j][:, 0 : 2 * H], func=AF.Copy,
                            )
                        else:
                            nc.scalar.activation(
                                out=sendbuf[:, 2 * j, :], in_=snd_ps[j][:, 0:H],
                                func=AF.Copy,
                            )
                nc.gpsimd.dma_start(
                    out=a2a_in[:].rearrange("(b p) e -> p b e", p=P),
                    in_=sendbuf[:, 0:SBT, :],
                )
                a2a_out = dr.tile([S, H], BF16)
                nc.gpsimd.collective_compute(
                    "AllToAll", ALU.bypass, replica_groups=rg,
                    ins=[a2a_in[:].opt()], outs=[a2a_out[:].opt()],
                )
                h1srcT = cp.tile([P, 2, e_pad], BF16)
                nc.gpsimd.dma_gather(
                    out_ap=h1srcT[:], in_ap=a2a_out[:], idxs_ap=h1src_sb[:],
                    num_idxs=e_pad, num_idxs_reg=e_pad, elem_size=H,
                    transpose=True, single_packet=False,
                )
                # h1shT via PE transposes of h1sb (PE is idle during the
                # AllToAll; alternating psum tags pipeline transpose+copy)
                h1shT = cp.tile([P, 2, NSH], BF16)
                for n in range(NT):
                    for oh in range(2):
                        tsh = psA.tile(
                            [P, P], BF16, space="PSUM", tag=f"agg{(n * 2 + oh) % 2}",
                            name=f"tsh_{n}_{oh}",
                        )
                        nc.tensor.transpose(
                            out=tsh[:], in_=h1sb[:, n, P * oh : P * (oh + 1)],
                            identity=ident_sb[:],
                        )
                        nc.scalar.activation(
                            out=h1shT[:, oh, P * n : P * (n + 1)], in_=tsh[:],
                            func=AF.Copy,
                        )
                # rotated copies for s=1..3: h1rot_r[p,c] = feat[(128c+p+32r)%256]
                # (32-partition aligned chunks -- walrus rejects unaligned
                # partition-offset spans)
                h1rots = [h1srcT]
                for r in range(1, 4):
                    h1r = cp.tile([P, 2, e_pad], BF16, name=f"h1rot{r}")
                    for c in range(2):
                        for d in range(4):
                            t = 32 * (d + r)
                            q, slot = t % P, (c if t < P else 1 - c)
                            nc.scalar.activation(
                                out=h1r[32 * d : 32 * d + 32, c, :],
                                in_=h1srcT[q : q + 32, slot, :], func=AF.Copy,
                            )
                    h1rots.append(h1r)

                if upto == "h1srcT":
                    d1 = dbg_out("d_h1srcT", [P, 2 * e_pad])
                    tmp = wp.tile([P, 2, e_pad], F32, tag="dbgf")
                    nc.vector.tensor_copy(out=tmp[:], in_=h1srcT[:])
                    nc.sync.dma_start(
                        out=d1[:].rearrange("p (c e) -> p c e", c=2), in_=tmp[:]
                    )

                # ======== conv2: 64 blocks, s-major (s=0 first)
                msg2_ps = [
                    psA.tile([P, 2 * H], F32, space="PSUM", tag=f"msg{j}", name=f"msg2_{j}")
                    for j in range((ET + 1) // 2)
                ]

                def m2(e):
                    return msg2_ps[e // 2][:, (e % 2) * H : (e % 2) * H + H]

                if not zb2:
                    for e in range(ET):
                        for ih in range(2):
                            nc.tensor.matmul(
                                m2(e), lhsT=h1srcT[:, ih, P * e : P * (e + 1)],
                                rhs=b2p_sb[:, ih, :], start=(ih == 0 and e % 2 == 0),
                                stop=False, skip_group_check=True,
                            )
                for b in range(64):
                    s2, q2, ih = b // 16, (b % 16) // 2, b % 2
                    srct = h1rots[s2]
                    zt = wp.tile([P, e_pad], BF16, tag="zt", bufs=4)
                    nc.vector.tensor_tensor(
                        out=zt[:], in0=srct[:, ih, :], in1=bcq_sb[:, q2, :], op=ALU.mult
                    )
                    for e in range(ET):
                        nc.tensor.matmul(
                            m2(e), lhsT=zt[:, P * e : P * (e + 1)], rhs=w2p_sb[:, b, :],
                            start=(zb2 and b == 0 and e % 2 == 0), stop=(b == 63),
                            skip_group_check=True,
                        )

                agg2_ps = [
                    psA.tile([P, 2 * H], F32, space="PSUM", tag=f"agg{j}", name=f"agg2_{j}")
                    for j in range(NT // 2)
                ]

                def a2(n):
                    return agg2_ps[n // 2][:, (n % 2) * H : (n % 2) * H + H]

                msbs2 = []
                for j in range((ET + 1) // 2):
                    w = min(2 * H, (ET - 2 * j) * H)
                    msb = wp.tile([P, 2 * H], BF16, tag="msb", bufs=5)
                    nc.scalar.activation(out=msb[:, 0:w], in_=msg2_ps[j][:, 0:w], func=AF.Copy)
                    msbs2.append(msb)

                def root2(n):
                    return [
                        (h1shT[:, kh, P * n : P * (n + 1)], r2wb_sb[:, kh, :])
                        for kh in range(2)
                    ]

                scatter_root(a2, msbs2, root2, None if zb2 else b2sbb_sb[:])

                h2sb = cp.tile([P, NT, H], BF16)
                for j in range(NT // 2):
                    nc.scalar.activation(
                        out=h2sb[:, 2 * j : 2 * j + 2, :], in_=agg2_ps[j][:, 0 : 2 * H],
                        func=AF.Copy,
                    )

                if upto == "h2":
                    dh = dbg_out("d_h2", [P, NT * H])
                    tmp = wp.tile([P, NT, H], F32, tag="dbgf")
                    nc.vector.tensor_copy(out=tmp[:], in_=h2sb[:])
                    nc.sync.dma_start(
                        out=dh[:].rearrange("p (t o) -> p t o", o=H), in_=tmp[:]
                    )

                # ======== pool (transposed, recip folded into scp) + z1T partials
                # meanT_ps[:, oh, g*128:...] = sum_n h2sb[:,n,128oh:].T @ scp_blk(n,g)
                meanT_ps = psA.tile([P, 2, H], F32, space="PSUM", tag="agg0", name="meanT")
                for n in range(NT):
                    for oh in range(2):
                        for g in range(GT):
                            blk = n * GT + g
                            nc.tensor.matmul(
                                meanT_ps[:, oh, P * g : P * (g + 1)],
                                lhsT=h2sb[:, n, P * oh : P * (oh + 1)],
                                rhs=scp_sb[:, P * blk : P * (blk + 1)],
                                start=(n == 0 and oh == 0 and g == 0),
                                stop=(n == NT - 1 and oh == 1 and g == GT - 1),
                                skip_group_check=True,
                            )
                meanT_sb = cp.tile([P, 2, H], BF16)
                nc.scalar.activation(out=meanT_sb[:], in_=meanT_ps[:], func=AF.Copy)
                # z1T[g, m] = sum_h meanT[h, g] * l1w[h, m]  (+ l1b/8 via ones row)
                z1T_ps = psA.tile([P, GT, H // 2], F32, space="PSUM", tag="agg1", name="z1T")
                for g in range(GT):
                    for oh in range(2):
                        nc.tensor.matmul(
                            z1T_ps[:, g, :],
                            lhsT=meanT_sb[:, oh, P * g : P * (g + 1)],
                            rhs=l1wb_sb[:, oh, :],
                            start=(g == 0 and oh == 0),
                            stop=(zl1 and g == GT - 1 and oh == 1),
                            skip_group_check=True,
                        )
                    if not zl1:
                        nc.tensor.matmul(
                            z1T_ps[:, g, :], lhsT=ones_sb[:], rhs=l1brow_sb[:],
                            start=False, stop=(g == GT - 1), skip_group_check=True,
                        )
                z1T = cp.tile([P, GT, H // 2], F32)
                nc.vector.tensor_copy(out=z1T[:], in_=z1T_ps[:])
                rs_in = dr.tile([N_GRAPHS, H // 2], F32)
                nc.sync.dma_start(
                    out=rs_in[:].rearrange("(g p) m -> p g m", p=P), in_=z1T[:]
                )

            # ======== tail: ReduceScatter, local readout, AllGather
            with tc.tile_pool(name="psB", bufs=1, space="PSUM") as psB:
                rs_out = dr.tile([GSH, H // 2], F32)
                nc.gpsimd.collective_compute(
                    "ReduceScatter", ALU.add, replica_groups=rg,
                    ins=[rs_in[:].opt()], outs=[rs_out[:].opt()],
                )
                # ======== local readout of GSH graphs
                rs_sb = cp.tile([GSH, H // 2], F32)
                nc.sync.dma_start(out=rs_sb[:], in_=rs_out[:])
                # fused relu(x) * l2w with free-dim reduction in one DVE op
                prod = wp.tile([GSH, H // 2], F32, tag="t2")
                red = wp.tile([GSH, 1], F32, tag="t3")
                nc.vector.scalar_tensor_tensor(
                    out=prod[:], in0=rs_sb[:], scalar=0.0, in1=l2w_sb[:],
                    op0=ALU.max, op1=ALU.mult, accum_out=red[:],
                )
                osb = wp.tile([GSH, 1], F32, tag="t4")
                nc.scalar.activation(
                    out=osb[:], in_=red[:], func=AF.Sigmoid, bias=l2b_sb[:, 0:1]
                )
                ag_in = dr.tile([GSH, 1], F32)
                nc.sync.dma_start(out=ag_in[:], in_=osb[:])
                ag_out = dr.tile([N_GRAPHS, 1], F32, addr_space="Shared")
                nc.gpsimd.collective_compute(
                    "AllGather", ALU.bypass, replica_groups=rg,
                    ins=[ag_in[:].opt()], outs=[ag_out[:].opt()],
                )
                nc.sync.dma_start(out=out[:], in_=ag_out[:])

    nc.compile()
    return nc


def _prep_inputs(inputs):
    x = np.asarray(inputs["x"], dtype=np.float32)
    ei = np.asarray(inputs["edge_index"])
    attr = np.asarray(inputs["edge_attr"], dtype=np.float32)
    batch = np.asarray(inputs["batch"]).astype(np.int64)
    src, dst = ei[0].astype(np.int64), ei[1].astype(np.int64)

    owner = dst // NSH
    per_core = []
    for c in range(NCORES):
        eids = np.nonzero(owner == c)[0]
        eids = eids[np.argsort(dst[eids], kind="stable")]
        per_core.append(eids)
    need = max(max(len(e) for e in per_core), 1)
    e_pad = max(((need + P - 1) // P) * P, P)
    ET = e_pad // P

    # static union of scatter blocks (e_tile, n_tile)
    blocks = set()
    for c in range(NCORES):
        dstl = dst[per_core[c]] - c * NSH
        for e in range(ET):
            seg = dstl[e * P : (e + 1) * P]
            if len(seg) == 0:
                continue
            for n in range(int(seg.min()) // P, int(seg.max()) // P + 1):
                blocks.add((e, int(n)))
    sc_blocks = sorted(blocks)
    NSC = len(sc_blocks)

    # A2A send rows (dedup per (sender c, receiver d) pair) and receive mapping
    send_rows = [[None] * NCORES for _ in range(NCORES)]
    recv_pos_parts = [[None] * NCORES for _ in range(NCORES)]  # [d][c]
    maxrows = 1
    for d in range(NCORES):
        eids = per_core[d]
        srcs = src[eids]
        co = srcs // NSH
        for c in range(NCORES):
            mask = co == c
            uniq, inv = np.unique(srcs[mask] - c * NSH, return_inverse=True)
            send_rows[c][d] = uniq
            recv_pos_parts[d][c] = (np.nonzero(mask)[0], inv)
            maxrows = max(maxrows, len(uniq))
    SB = ((maxrows + 15) // 16) * 16
    S = NCORES * SB

    # host-permuted weights (shared)
    nn1_w = np.asarray(inputs["nn1_w"], np.float32)  # [32, 64*256]
    nn2_w = np.asarray(inputs["nn2_w"], np.float32)  # [32, 256*256]
    pidx = np.arange(P)
    g32 = pidx // 32
    j32 = pidx % 32
    nn1_r = nn1_w.reshape(DE, DN, H)
    w1p = np.zeros((P, 16, H), np.float32)
    for t in range(16):
        q, s = t // 2, t % 2
        k = 4 * q + g32
        i = (32 * (g32 + s) + j32) % DN
        w1p[:, t, :] = nn1_r[k, i, :]
    w1p = w1p.astype(BF)
    nn2_r = nn2_w.reshape(DE, H, H)
    w2p = np.zeros((P, 64, H), np.float32)
    for b in range(64):
        s, q, ih = b // 16, (b % 16) // 2, b % 2
        k = 4 * q + g32
        i = (128 * ih + 32 * (g32 + s) + j32) % H
        w2p[:, b, :] = nn2_r[k, i, :]
    w2p = w2p.astype(BF)

    nn1_b = np.asarray(inputs["nn1_b"], np.float32).reshape(DN, H)
    nn2_b = np.asarray(inputs["nn2_b"], np.float32).reshape(H, H)
    b2p = np.stack([nn2_b[0:P, :], nn2_b[P : 2 * P, :]], axis=1)  # [128, 2, 256]
    r1w = np.asarray(inputs["root1_w"], np.float32)
    bias1 = np.asarray(inputs["bias1"], np.float32)
    r1wb = np.concatenate([r1w, bias1.reshape(1, H)], axis=0)  # [65, 256]
    r2w = np.asarray(inputs["root2_w"], np.float32)
    r2wb = np.stack([r2w[0:P, :], r2w[P : 2 * P, :]], axis=1)  # [128, 2, 256]
    bias2 = np.asarray(inputs["bias2"], np.float32).reshape(1, H)
    l1w = np.asarray(inputs["lin1_w"], np.float32)  # [256, 128]
    l1wb = np.stack([l1w[0:P, :], l1w[P : 2 * P, :]], axis=1)  # [128, 2, 128]
    l1b = np.asarray(inputs["lin1_b"], np.float32).reshape(1, H // 2)
    l2w = np.asarray(inputs["lin2_w"], np.float32).reshape(1, H // 2)
    l2b = np.asarray(inputs["lin2_b"], np.float32).reshape(1, 1)
    GSH = N_GRAPHS // NCORES

    cnt = np.bincount(batch, minlength=N_GRAPHS).astype(np.float32)
    recip_g = 1.0 / np.maximum(cnt, 1.0)  # [256], per graph

    common = {
        "w1p": w1p, "w2p": w2p,
        "b1p": nn1_b.astype(BF), "b2p": b2p.astype(BF),
        "r1wb": r1wb.astype(BF), "r2wb": r2wb.astype(BF),
        "b2sbb": bias2.astype(BF),
        "l1wb": l1wb.astype(BF), "l1brow": (l1b / NCORES).astype(BF),
        "l2wrep": np.tile(l2w, (GSH, 1)).astype(np.float32),
        "l2brep": np.tile(l2b, (GSH, 1)).astype(np.float32),
        "identb": np.eye(P, dtype=BF),
    }

    in_maps = []
    for c in range(NCORES):
        eids = per_core[c]
        ne = len(eids)
        srcs = src[eids]
        dstl = (dst[eids] - c * NSH).astype(np.int64)

        xg = x[srcs, :].astype(BF)  # [ne, 64]
        xsrc2 = np.zeros((P, 2, e_pad), BF)
        for s in range(2):
            iofs = (32 * (g32 + s) + j32) % DN  # [128]
            xsrc2[:, s, 0:ne] = xg[:, iofs].T

        ag = attr[eids, :]  # [ne, 32]
        bcq = np.zeros((P, 8, e_pad), BF)
        for q in range(8):
            for g in range(4):
                bcq[32 * g : 32 * g + 32, q, 0:ne] = ag[:, 4 * q + g].astype(BF)[None, :]

        scm = np.zeros((P, NSC * P), BF)
        for bi, (e, n) in enumerate(sc_blocks):
            seg = dstl[e * P : min((e + 1) * P, ne)]
            for p, dv in enumerate(seg):
                q = dv - n * P
                if 0 <= q < P:
                    scm[p, bi * P + q] = 1.0

        batch_l = batch[c * NSH : (c + 1) * NSH]
        scp = np.zeros((P, NT * GT * P), BF)
        for n in range(NT):
            for g in range(GT):
                blk = n * GT + g
                bseg = batch_l[n * P : (n + 1) * P]
                for p, bv in enumerate(bseg):
                    q = bv - g * P
                    if 0 <= q < P:
                        scp[p, blk * P + q] = BF(recip_g[bv])

        xshT = np.ones((DN + 1, NSH), BF)
        xshT[0:DN, :] = x[c * NSH : (c + 1) * NSH, :].astype(BF).T

        snd_idx = np.full(S, -1, np.int64)
        for d in range(NCORES):
            rows = send_rows[c][d]
            snd_idx[d * SB : d * SB + len(rows)] = rows
        SBT = S // P
        selm = np.zeros((P, SBT * NT * P), BF)
        for row in range(S):
            v = snd_idx[row]
            if v < 0:
                continue
            r, q = row // P, row % P
            nt_, npart = int(v) // P, int(v) % P
            selm[npart, (r * NT + nt_) * P + q] = 1.0
        h1src_idx = np.zeros(e_pad, np.int16)
        for d2 in range(NCORES):
            pos, inv = recv_pos_parts[c][d2]
            h1src_idx[pos] = d2 * SB + inv

        m = dict(common)
        m["xsrc2"] = xsrc2
        m["bcq"] = bcq
        m["scm"] = scm
        m["scp"] = scp
        m["sel"] = selm
        m["xshT"] = xshT
        m["h1src_w"] = _wrap_idx(h1src_idx, e_pad)
        in_maps.append(m)

    zb = (
        bool(np.all(np.asarray(inputs["nn1_b"]) == 0)),
        bool(np.all(np.asarray(inputs["nn2_b"]) == 0))
        and bool(np.all(np.asarray(inputs["bias2"]) == 0)),
        bool(np.all(np.asarray(inputs["lin1_b"]) == 0)),
    )
    _PREP["args"] = (e_pad, S, tuple(sc_blocks), zb)
    return e_pad, in_maps


def kernel(**inputs) -> np.ndarray:
    e_pad, in_maps = _prep_inputs(inputs)
    if e_pad not in _cache:
        ep, S, blocks, zb = _PREP["args"]
        _cache[e_pad] = _build(ep, S, list(blocks), zb=zb)
    nc = _cache[e_pad]
    res = bass_utils.run_bass_kernel_spmd(nc, in_maps, core_ids=list(range(NCORES)))
    return np.asarray(res.results[0]["out"], dtype=np.float32)


def run_debug(upto, **inputs):
    e_pad, in_maps = _prep_inputs(inputs)
    ep, S, blocks, zb = _PREP["args"]
    nc = _build(ep, S, list(blocks), zb=zb, upto=upto)
    res = bass_utils.run_bass_kernel_spmd(nc, in_maps, core_ids=list(range(NCORES)))
    return e_pad, res


# revision 39
# speedup vs baseline: 1.0588x; 1.0021x over previous
"""Trainium2 Bass kernel for nn_NNModel2 (2x NNConv GNN + pooled MLP readout).

Self-contained: accepts FULL inputs, shards edges across 8 NeuronCores
(edge-parallel by dst owner), returns the FULL [256, 1] output.

v2 design:
  - All gathers/transposes/broadcasts of *input-derived* data are done on the
    HOST and fed as per-core tensors (bf16): xsrcT, bcp (pair-broadcast attr),
    scatter one-hot matrices, permuted edge-MLP weights.
  - conv layer z-trick: z[e,(k,i)] = attr[e,k]*x[src,i]; msg = z @ W' done as
    PSUM-accumulated matmuls over 128-row (k,i) blocks. attr broadcast uses
    PAIR tiles (k0 on partitions 0:64, k1 on 64:128); conv2 covers full i-range
    with a partition-rotated copy of h1srcT (s=1 blocks).
  - h1 exchange via AllToAll of per-edge-needed rows (deduped per (src-owner,
    dst-owner) pair) instead of AllGather: ~0.7MB vs 2MB collective payload.
  - Tail: z1 partials computed locally, ReduceScatter over graphs, local
    readout of 32 graphs/core, AllGather of [256,1] result.
"""

import sys

sys.path.insert(0, "/opt/trn_rl_repo")

import numpy as np
import ml_dtypes

from concourse import bacc, bass, mybir
import concourse.tile as tile
from concourse import bass_utils

P = 128
NCORES = 8
N_NODES = 4096
N_EDGES = 8192
N_GRAPHS = 256
DN = 64
DE = 32
H = 256
NSH = N_NODES // NCORES  # 512
NT = NSH // P  # 4
GT = N_GRAPHS // P  # 2

F32 = mybir.dt.float32
BF16 = mybir.dt.bfloat16
I16 = mybir.dt.int16
AF = mybir.ActivationFunctionType
ALU = mybir.AluOpType
BF = ml_dtypes.bfloat16

_cache = {}
_PREP = {}


def _wrap_idx(idx, n):
    idx = np.asarray(idx, dtype=np.int16)
    assert idx.shape == (n,) and n % 16 == 0
    return np.tile(idx.reshape(n // 16, 16).T, (8, 1)).copy()


def _build(e_pad, S, sc_blocks, zb=(False, False, False), upto="full"):
    ET = e_pad // P
    SBT = S // P  # send-buffer tiles
    nc = bacc.Bacc(num_devices=NCORES)

    # ---- per-core inputs (host-prepped)
    xsrc2 = nc.dram_tensor("xsrc2", [P, 2, e_pad], BF16, kind="ExternalInput")
    bcq = nc.dram_tensor("bcq", [P, 8, e_pad], BF16, kind="ExternalInput")
    scm = nc.dram_tensor("scm", [P, len(sc_blocks) * P], BF16, kind="ExternalInput")
    scp = nc.dram_tensor("scp", [P, NT * GT * P], BF16, kind="ExternalInput")
    sel = nc.dram_tensor("sel", [P, (S // P) * NT * P], BF16, kind="ExternalInput")
    xshT = nc.dram_tensor("xshT", [DN + 1, NSH], BF16, kind="ExternalInput")
    h1src_w = nc.dram_tensor("h1src_w", [P, e_pad // 16], I16, kind="ExternalInput")
    identb = nc.dram_tensor("identb", [P, P], BF16, kind="ExternalInput")
    # ---- shared weights (host-permuted, bf16)
    w1p = nc.dram_tensor("w1p", [P, 16, H], BF16, kind="ExternalInput")
    w2p = nc.dram_tensor("w2p", [P, 64, H], BF16, kind="ExternalInput")
    b1p = nc.dram_tensor("b1p", [DN, H], BF16, kind="ExternalInput")
    b2p = nc.dram_tensor("b2p", [P, 2, H], BF16, kind="ExternalInput")
    r1wb = nc.dram_tensor("r1wb", [DN + 1, H], BF16, kind="ExternalInput")
    r2wb = nc.dram_tensor("r2wb", [P, 2, H], BF16, kind="ExternalInput")
    b2sbb = nc.dram_tensor("b2sbb", [1, H], BF16, kind="ExternalInput")
    l1wb = nc.dram_tensor("l1wb", [P, 2, H // 2], BF16, kind="ExternalInput")
    l1brow = nc.dram_tensor("l1brow", [1, H // 2], BF16, kind="ExternalInput")
    l2wrep = nc.dram_tensor("l2wrep", [N_GRAPHS // NCORES, H // 2], F32, kind="ExternalInput")
    l2brep = nc.dram_tensor("l2brep", [N_GRAPHS // NCORES, 1], F32, kind="ExternalInput")
    out = nc.dram_tensor("out", [N_GRAPHS, 1], F32, kind="ExternalOutput")

    def dbg_out(name, shape):
        return nc.dram_tensor(name, shape, F32, kind="ExternalOutput")

    zb1, zb2, zl1 = zb
    rg = [list(range(NCORES))]
    NSC = len(sc_blocks)
    GSH = N_GRAPHS // NCORES  # 32 graphs per core in the tail

    # first bank-touch bookkeeping for agg scatter (bank = n // 2)
    first_touch = {}
    for bi, (e, n) in enumerate(sc_blocks):
        first_touch.setdefault(n // 2, ("sc", bi))
    for n in range(NT):
        first_touch.setdefault(n // 2, ("root", n))

    with tile.TileContext(nc, num_cores=NCORES) as tc:
        with (
            tc.tile_pool(name="const", bufs=1) as cp,
            tc.tile_pool(name="work", bufs=3) as wp,
            tc.tile_pool(name="dram", bufs=1, space="DRAM") as dr,
        ):
            # ======== stage A: loads (SP queue), conv1-critical first.
            # Same-queue DMA transfers start in issue order, so priority ==
            # issue order here.
            bcq_sb = cp.tile([P, 8, e_pad], BF16)
            nc.sync.dma_start(out=bcq_sb[:, 0:2, :], in_=bcq[:, 0:2, :])
            xsrc2_sb = cp.tile([P, 2, e_pad], BF16)
            nc.sync.dma_start(out=xsrc2_sb[:, 0:1, :], in_=xsrc2[:, 0:1, :])
            w1p_sb = cp.tile([P, 16, H], BF16)
            nc.sync.dma_start(out=w1p_sb[:, 0:4, :], in_=w1p[:, 0:4, :])
            nc.sync.dma_start(out=xsrc2_sb[:, 1:2, :], in_=xsrc2[:, 1:2, :])
            b1p_sb = cp.tile([DN, H], BF16)
            nc.sync.dma_start(out=b1p_sb[:], in_=b1p[:])
            for c in range(1, 4):
                nc.sync.dma_start(
                    out=bcq_sb[:, 2 * c : 2 * c + 2, :], in_=bcq[:, 2 * c : 2 * c + 2, :]
                )
                if c == 1:
                    nc.sync.dma_start(out=w1p_sb[:, 4:8, :], in_=w1p[:, 4:8, :])
                if c == 2:
                    nc.sync.dma_start(out=w1p_sb[:, 8:16, :], in_=w1p[:, 8:16, :])
            scm_sb = cp.tile([P, NSC * P], BF16)
            nc.sync.dma_start(out=scm_sb[:], in_=scm[:])
            xshT_sb = cp.tile([DN + 1, NSH], BF16)
            nc.sync.dma_start(out=xshT_sb[:], in_=xshT[:])
            r1wb_sb = cp.tile([DN + 1, H], BF16)
            nc.sync.dma_start(out=r1wb_sb[:], in_=r1wb[:])
            sel_sb = cp.tile([P, (S // P) * NT * P], BF16)
            nc.sync.dma_start(out=sel_sb[:], in_=sel[:])
            h1src_sb = cp.tile([P, e_pad // 16], I16)
            nc.sync.dma_start(out=h1src_sb[:], in_=h1src_w[:])
            ident_sb = cp.tile([P, P], BF16)
            nc.sync.dma_start(out=ident_sb[:], in_=identb[:])
            # conv2/tail loads last (small ones first, then the big w2p)
            a2a_in = dr.tile([S, H], BF16)
            b2p_sb = cp.tile([P, 2, H], BF16)
            nc.sync.dma_start(out=b2p_sb[:], in_=b2p[:])
            r2wb_sb = cp.tile([P, 2, H], BF16)
            nc.sync.dma_start(out=r2wb_sb[:], in_=r2wb[:])
            b2sbb_sb = cp.tile([1, H], BF16)
            nc.sync.dma_start(out=b2sbb_sb[:], in_=b2sbb[:])
            scp_sb = cp.tile([P, NT * GT * P], BF16)
            nc.sync.dma_start(out=scp_sb[:], in_=scp[:])
            l1wb_sb = cp.tile([P, 2, H // 2], BF16)
            nc.sync.dma_start(out=l1wb_sb[:], in_=l1wb[:])
            l1brow_sb = cp.tile([1, H // 2], BF16)
            nc.sync.dma_start(out=l1brow_sb[:], in_=l1brow[:])
            l2w_sb = cp.tile([GSH, H // 2], F32)
            nc.sync.dma_start(out=l2w_sb[:], in_=l2wrep[:])
            l2b_sb = cp.tile([GSH, 1], F32)
            nc.sync.dma_start(out=l2b_sb[:], in_=l2brep[:])
            w2p_sb = cp.tile([P, 64, H], BF16)
            for c in range(4):
                nc.sync.dma_start(
                    out=w2p_sb[:, 16 * c : 16 * c + 16, :],
                    in_=w2p[:, 16 * c : 16 * c + 16, :],
                )

            with tc.tile_pool(name="psA", bufs=1, space="PSUM") as psA:
                # ======== conv1
                msg_ps = [
                    psA.tile([P, 2 * H], F32, space="PSUM", tag=f"msg{j}", name=f"msg1_{j}")
                    for j in range((ET + 1) // 2)
                ]

                def m1(e):
                    return msg_ps[e // 2][:, (e % 2) * H : (e % 2) * H + H]

                msbs = []

                zts1 = []
                for t in range(16):
                    q1, s1 = t // 2, t % 2
                    zt = wp.tile([P, e_pad], BF16, tag=f"zt1_{t}", name=f"zt1_{t}", bufs=1)
                    nc.vector.tensor_tensor(
                        out=zt[:], in0=xsrc2_sb[:, s1, :], in1=bcq_sb[:, q1, :],
                        op=ALU.mult,
                    )
                    zts1.append(zt)
                # e-major accumulation: each msg bank closes early so its
                # PSUM->SBUF copy overlaps the remaining matmuls
                for e in range(ET):
                    if not zb1:
                        nc.tensor.matmul(
                            m1(e), lhsT=xsrc2_sb[0:DN, 0, P * e : P * (e + 1)],
                            rhs=b1p_sb[:], start=(e % 2 == 0), stop=False,
                            skip_group_check=True,
                        )
                    for t in range(16):
                        nc.tensor.matmul(
                            m1(e), lhsT=zts1[t][:, P * e : P * (e + 1)],
                            rhs=w1p_sb[:, t, :],
                            start=(zb1 and t == 0 and e % 2 == 0), stop=(t == 15),
                            skip_group_check=True,
                        )
                    if e % 2 == 1 or e == ET - 1:
                        j = e // 2
                        w = min(2 * H, (ET - 2 * j) * H)
                        msb = wp.tile([P, 2 * H], BF16, tag="msb", bufs=5, name=f"msb1_{j}")
                        nc.scalar.activation(
                            out=msb[:, 0:w], in_=msg_ps[j][:, 0:w], func=AF.Copy
                        )
                        msbs.append(msb)

                agg_ps = [
                    psA.tile([P, 2 * H], F32, space="PSUM", tag=f"agg{j}", name=f"agg1_{j}")
                    for j in range(NT // 2)
                ]

                def a1(n):
                    return agg_ps[n // 2][:, (n % 2) * H : (n % 2) * H + H]


                ones_sb = cp.tile([1, P], BF16)
                nc.vector.memset(ones_sb[:], 1.0)

                def scatter_root(aget, msbs_l, root_lhs, bias_rhs):
                    for bi, (e, n) in enumerate(sc_blocks):
                        nc.tensor.matmul(
                            aget(n), lhsT=scm_sb[:, P * bi : P * (bi + 1)],
                            rhs=msbs_l[e // 2][:, (e % 2) * H : (e % 2) * H + H],
                            start=(first_touch[n // 2] == ("sc", bi)), stop=False,
                            skip_group_check=True,
                        )
                    for n in range(NT):
                        pairs = root_lhs(n)
                        for li, (lhs, rhs) in enumerate(pairs):
                            last = bias_rhs is None and li == len(pairs) - 1
                            nc.tensor.matmul(
                                aget(n), lhsT=lhs, rhs=rhs,
                                start=(first_touch[n // 2] == ("root", n) and li == 0),
                                stop=last, skip_group_check=True,
                            )
                        if bias_rhs is not None:
                            nc.tensor.matmul(
                                aget(n), lhsT=ones_sb[:], rhs=bias_rhs,
                                start=False, stop=True, skip_group_check=True,
                            )

                def root1(n):
                    return [(xshT_sb[:, P * n : P * (n + 1)], r1wb_sb[:])]

                # bias1 is folded into r1wb (row 64 = ones in xshT)
                scatter_root(a1, msbs, root1, None)

                h1sb = cp.tile([P, NT, H], BF16)
                for j in range(NT // 2):
                    nc.scalar.activation(
                        out=h1sb[:, 2 * j : 2 * j + 2, :], in_=agg_ps[j][:, 0 : 2 * H],
                        func=AF.Relu,
                    )

                if upto == "h1":
                    dh = dbg_out("d_h1", [P, NT * H])
                    tmp = wp.tile([P, NT, H], F32, tag="dbgf")
                    nc.vector.tensor_copy(out=tmp[:], in_=h1sb[:])
                    nc.sync.dma_start(
                        out=dh[:].rearrange("p (t o) -> p t o", o=H), in_=tmp[:]
                    )

                # ======== exchange: sendbuf rows via one-hot matmuls -> AllToAll
                snd_ps = [
                    psA.tile([P, 2 * H], F32, space="PSUM", tag=f"msg{j}", name=f"snd_{j}")
                    for j in range((SBT + 1) // 2)
                ]

                def sb_ps(r):
                    return snd_ps[r // 2][:, (r % 2) * H : (r % 2) * H + H]

                sendbuf = cp.tile([P, 2 * ((SBT + 1) // 2), H], BF16)
                for r in range(SBT):
                    for n in range(NT):
                        blk = r * NT + n
                        nc.tensor.matmul(
                            sb_ps(r), lhsT=sel_sb[:, P * blk : P * (blk + 1)],
                            rhs=h1sb[:, n, :], start=(n == 0 and r % 2 == 0),
                            stop=(n == NT - 1), skip_group_check=True,
                        )
                    if r % 2 == 1 or r == SBT - 1:
                        j = r // 2
                        if (SBT - 2 * j) >= 2:
                            nc.scalar.activation(
                                out=sendbuf[:, 2 * j : 2 * j + 2, :],
                                in_=snd_ps[j][:, 0 : 2 * H], func=AF.Copy,
                            )
                        else:
                            nc.scalar.activation(
                                out=sendbuf[:, 2 * j, :], in_=snd_ps[j][:, 0:H],
                                func=AF.Copy,
                            )
                nc.gpsimd.dma_start(
                    out=a2a_in[:].rearrange("(b p) e -> p b e", p=P),
                    in_=sendbuf[:, 0:SBT, :],
                )
                a2a_out = dr.tile([S, H], BF16)
                nc.gpsimd.collective_compute(
                    "AllToAll", ALU.bypass, replica_groups=rg,
                    ins=[a2a_in[:].opt()], outs=[a2a_out[:].opt()],
                )
                h1srcT = cp.tile([P, 2, e_pad], BF16)
                nc.gpsimd.dma_gather(
                    out_ap=h1srcT[:], in_ap=a2a_out[:], idxs_ap=h1src_sb[:],
                    num_idxs=e_pad, num_idxs_reg=e_pad, elem_size=H,
                    transpose=True, single_packet=False,
                )
                # h1shT via PE transposes of h1sb (PE is idle during the
                # AllToAll; alternating psum tags pipeline transpose+copy)
                h1shT = cp.tile([P, 2, NSH], BF16)
                for n in range(NT):
                    for oh in range(2):
                        tsh = psA.tile(
                            [P, P], BF16, space="PSUM", tag=f"agg{(n * 2 + oh) % 2}",
                            name=f"tsh_{n}_{oh}",
                        )
                        nc.tensor.transpose(
                            out=tsh[:], in_=h1sb[:, n, P * oh : P * (oh + 1)],
                            identity=ident_sb[:],
                        )
                        nc.scalar.activation(
                            out=h1shT[:, oh, P * n : P * (n + 1)], in_=tsh[:],
                            func=AF.Copy,
                        )
                # rotated copies for s=1..3: h1rot_r[p,c] = feat[(128c+p+32r)%256]
                # (32-partition aligned chunks -- walrus rejects unaligned
                # partition-offset spans)
                h1rots = [h1srcT]
                for r in range(1, 4):
                    h1r = cp.tile([P, 2, e_pad], BF16, name=f"h1rot{r}")
                    for c in range(2):
                        for d in range(4):
                            t = 32 * (d + r)
                            q, slot = t % P, (c if t < P else 1 - c)
                            nc.scalar.activation(
                                out=h1r[32 * d : 32 * d + 32, c, :],
                                in_=h1srcT[q : q + 32, slot, :], func=AF.Copy,
                            )
                    h1rots.append(h1r)

                if upto == "h1srcT":
                    d1 = dbg_out("d_h1srcT", [P, 2 * e_pad])
                    tmp = wp.tile([P, 2, e_pad], F32, tag="dbgf")
                    nc.vector.tensor_copy(out=tmp[:], in_=h1srcT[:])
                    nc.sync.dma_start(
                        out=d1[:].rearrange("p (c e) -> p c e", c=2), in_=tmp[:]
                    )

                # ======== conv2: 64 blocks, s-major (s=0 first)
                msg2_ps = [
                    psA.tile([P, 2 * H], F32, space="PSUM", tag=f"msg{j}", name=f"msg2_{j}")
                    for j in range((ET + 1) // 2)
                ]

                def m2(e):
                    return msg2_ps[e // 2][:, (e % 2) * H : (e % 2) * H + H]

                if not zb2:
                    for e in range(ET):
                        for ih in range(2):
                            nc.tensor.matmul(
                                m2(e), lhsT=h1srcT[:, ih, P * e : P * (e + 1)],
                                rhs=b2p_sb[:, ih, :], start=(ih == 0 and e % 2 == 0),
                                stop=False, skip_group_check=True,
                            )
                for b in range(64):
                    s2, q2, ih = b // 16, (b % 16) // 2, b % 2
                    srct = h1rots[s2]
                    zt = wp.tile([P, e_pad], BF16, tag="zt", bufs=4)
                    nc.vector.tensor_tensor(
                        out=zt[:], in0=srct[:, ih, :], in1=bcq_sb[:, q2, :], op=ALU.mult
                    )
                    for e in range(ET):
                        nc.tensor.matmul(
                            m2(e), lhsT=zt[:, P * e : P * (e + 1)], rhs=w2p_sb[:, b, :],
                            start=(zb2 and b == 0 and e % 2 == 0), stop=(b == 63),
                            skip_group_check=True,
                        )

                agg2_ps = [
                    psA.tile([P, 2 * H], F32, space="PSUM", tag=f"agg{j}", name=f"agg2_{j}")
                    for j in range(NT // 2)
                ]

                def a2(n):
                    return agg2_ps[n // 2][:, (n % 2) * H : (n % 2) * H + H]

                msbs2 = []
                for j in range((ET + 1) // 2):
                    w = min(2 * H, (ET - 2 * j) * H)
                    msb = wp.tile([P, 2 * H], BF16, tag="msb", bufs=5)
                    nc.scalar.activation(out=msb[:, 0:w], in_=msg2_ps[j][:, 0:w], func=AF.Copy)
                    msbs2.append(msb)

                def root2(n):
                    return [
                        (h1shT[:, kh, P * n : P * (n + 1)], r2wb_sb[:, kh, :])
                        for kh in range(2)
                    ]

                scatter_root(a2, msbs2, root2, None if zb2 else b2sbb_sb[:])

                h2sb = cp.tile([P, NT, H], BF16)
                for j in range(NT // 2):
                    nc.scalar.activation(
                        out=h2sb[:, 2 * j : 2 * j + 2, :], in_=agg2_ps[j][:, 0 : 2 * H],
                        func=AF.Copy,
                    )

                if upto == "h2":
                    dh = dbg_out("d_h2", [P, NT * H])
                    tmp = wp.tile([P, NT, H], F32, tag="dbgf")
                    nc.vector.tensor_copy(out=tmp[:], in_=h2sb[:])
                    nc.sync.dma_start(
                        out=dh[:].rearrange("p (t o) -> p t o", o=H), in_=tmp[:]
                    )

                # ======== pool (transposed, recip folded into scp) + z1T partials
                # meanT_ps[:, oh, g*128:...] = sum_n h2sb[:,n,128oh:].T @ scp_blk(n,g)
                meanT_ps = psA.tile([P, 2, H], F32, space="PSUM", tag="agg0", name="meanT")
                for n in range(NT):
                    for oh in range(2):
                        for g in range(GT):
                            blk = n * GT + g
                            nc.tensor.matmul(
                                meanT_ps[:, oh, P * g : P * (g + 1)],
                                lhsT=h2sb[:, n, P * oh : P * (oh + 1)],
                                rhs=scp_sb[:, P * blk : P * (blk + 1)],
                                start=(n == 0 and oh == 0 and g == 0),
                                stop=(n == NT - 1 and oh == 1 and g == GT - 1),
                                skip_group_check=True,
                            )
                meanT_sb = cp.tile([P, 2, H], BF16)
                nc.scalar.activation(out=meanT_sb[:], in_=meanT_ps[:], func=AF.Copy)
                # z1T[g, m] = sum_h meanT[h, g] * l1w[h, m]  (+ l1b/8 via ones row)
                z1T_ps = psA.tile([P, GT, H // 2], F32, space="PSUM", tag="agg1", name="z1T")
                for g in range(GT):
                    for oh in range(2):
                        nc.tensor.matmul(
                            z1T_ps[:, g, :],
                            lhsT=meanT_sb[:, oh, P * g : P * (g + 1)],
                            rhs=l1wb_sb[:, oh, :],
                            start=(g == 0 and oh == 0),
                            stop=(zl1 and g == GT - 1 and oh == 1),
                            skip_group_check=True,
                        )
                    if not zl1:
                        nc.tensor.matmul(
                            z1T_ps[:, g, :], lhsT=ones_sb[:], rhs=l1brow_sb[:],
                            start=False, stop=(g == GT - 1), skip_group_check=True,
                        )
                z1T = cp.tile([P, GT, H // 2], F32)
                nc.vector.tensor_copy(out=z1T[:], in_=z1T_ps[:])
                rs_in = dr.tile([N_GRAPHS, H // 2], F32)
                nc.sync.dma_start(
                    out=rs_in[:].rearrange("(g p) m -> p g m", p=P), in_=z1T[:]
                )

            # ======== tail: ReduceScatter, local readout, AllGather
            with tc.tile_pool(name="psB", bufs=1, space="PSUM") as psB:
                rs_out = dr.tile([GSH, H // 2], F32)
                nc.gpsimd.collective_compute(
                    "ReduceScatter", ALU.add, replica_groups=rg,
                    ins=[rs_in[:].opt()], outs=[rs_out[:].opt()],
                )
                # ======== local readout of GSH graphs
                rs_sb = cp.tile([GSH, H // 2], F32)
                nc.sync.dma_start(out=rs_sb[:], in_=rs_out[:])
                # fused relu(x) * l2w with free-dim reduction in one DVE op
                prod = wp.tile([GSH, H // 2], F32, tag="t2")
                red = wp.tile([GSH, 1], F32, tag="t3")
                nc.vector.scalar_tensor_tensor(
                    out=prod[:], in0=rs_sb[:], scalar=0.0, in1=l2w_sb[:],
                    op0=ALU.max, op1=ALU.mult, accum_out=red[:],
                )
                osb = wp.tile([GSH, 1], F32, tag="t4")
                nc.scalar.activation(
                    out=osb[:], in_=red[:], func=AF.Sigmoid, bias=l2b_sb[:, 0:1]
                )
                ag_in = dr.tile([GSH, 1], F32)
                nc.sync.dma_start(out=ag_in[:], in_=osb[:])
                ag_out = dr.tile([N_GRAPHS, 1], F32, addr_space="Shared")
                nc.gpsimd.collective_compute(
                    "AllGather", ALU.bypass, replica_groups=rg,
                    ins=[ag_in[:].opt()], outs=[ag_out[:].opt()],
                )
                nc.sync.dma_start(out=out[:], in_=ag_out[:])

    nc.compile()
    return nc


def _prep_inputs(inputs):
    x = np.asarray(inputs["x"], dtype=np.float32)
    ei = np.asarray(inputs["edge_index"])
    attr = np.asarray(inputs["edge_attr"], dtype=np.float32)
    batch = np.asarray(inputs["batch"]).astype(np.int64)
    src, dst = ei[0].astype(np.int64), ei[1].astype(np.int64)

    owner = dst // NSH
    per_core = []
    for c in range(NCORES):
        eids = np.nonzero(owner == c)[0]
        eids = eids[np.argsort(dst[eids], kind="stable")]
        per_core.append(eids)
    need = max(max(len(e) for e in per_core), 1)
    e_pad = max(((need + P - 1) // P) * P, P)
    ET = e_pad // P

    # static union of scatter blocks (e_tile, n_tile)
    blocks = set()
    for c in range(NCORES):
        dstl = dst[per_core[c]] - c * NSH
        for e in range(ET):
            seg = dstl[e * P : (e + 1) * P]
            if len(seg) == 0:
                continue
            for n in range(int(seg.min()) // P, int(seg.max()) // P + 1):
                blocks.add((e, int(n)))
    sc_blocks = sorted(blocks)
    NSC = len(sc_blocks)

    # A2A send rows (dedup per (sender c, receiver d) pair) and receive mapping
    send_rows = [[None] * NCORES for _ in range(NCORES)]
    recv_pos_parts = [[None] * NCORES for _ in range(NCORES)]  # [d][c]
    maxrows = 1
    for d in range(NCORES):
        eids = per_core[d]
        srcs = src[eids]
        co = srcs // NSH
        for c in range(NCORES):
            mask = co == c
            uniq, inv = np.unique(srcs[mask] - c * NSH, return_inverse=True)
            send_rows[c][d] = uniq
            recv_pos_parts[d][c] = (np.nonzero(mask)[0], inv)
            maxrows = max(maxrows, len(uniq))
    SB = ((maxrows + 15) // 16) * 16
    S = NCORES * SB

    # host-permuted weights (shared)
    nn1_w = np.asarray(inputs["nn1_w"], np.float32)  # [32, 64*256]
    nn2_w = np.asarray(inputs["nn2_w"], np.float32)  # [32, 256*256]
    pidx = np.arange(P)
    g32 = pidx // 32
    j32 = pidx % 32
    nn1_r = nn1_w.reshape(DE, DN, H)
    w1p = np.zeros((P, 16, H), np.float32)
    for t in range(16):
        q, s = t // 2, t % 2
        k = 4 * q + g32
        i = (32 * (g32 + s) + j32) % DN
        w1p[:, t, :] = nn1_r[k, i, :]
    w1p = w1p.astype(BF)
    nn2_r = nn2_w.reshape(DE, H, H)
    w2p = np.zeros((P, 64, H), np.float32)
    for b in range(64):
        s, q, ih = b // 16, (b % 16) // 2, b % 2
        k = 4 * q + g32
        i = (128 * ih + 32 * (g32 + s) + j32) % H
        w2p[:, b, :] = nn2_r[k, i, :]
    w2p = w2p.astype(BF)

    nn1_b = np.asarray(inputs["nn1_b"], np.float32).reshape(DN, H)
    nn2_b = np.asarray(inputs["nn2_b"], np.float32).reshape(H, H)
    b2p = np.stack([nn2_b[0:P, :], nn2_b[P : 2 * P, :]], axis=1)  # [128, 2, 256]
    r1w = np.asarray(inputs["root1_w"], np.float32)
    bias1 = np.asarray(inputs["bias1"], np.float32)
    r1wb = np.concatenate([r1w, bias1.reshape(1, H)], axis=0)  # [65, 256]
    r2w = np.asarray(inputs["root2_w"], np.float32)
    r2wb = np.stack([r2w[0:P, :], r2w[P : 2 * P, :]], axis=1)  # [128, 2, 256]
    bias2 = np.asarray(inputs["bias2"], np.float32).reshape(1, H)
    l1w = np.asarray(inputs["lin1_w"], np.float32)  # [256, 128]
    l1wb = np.stack([l1w[0:P, :], l1w[P : 2 * P, :]], axis=1)  # [128, 2, 128]
    l1b = np.asarray(inputs["lin1_b"], np.float32).reshape(1, H // 2)
    l2w = np.asarray(inputs["lin2_w"], np.float32).reshape(1, H // 2)
    l2b = np.asarray(inputs["lin2_b"], np.float32).reshape(1, 1)
    GSH = N_GRAPHS // NCORES

    cnt = np.bincount(batch, minlength=N_GRAPHS).astype(np.float32)
    recip_g = 1.0 / np.maximum(cnt, 1.0)  # [256], per graph

    common = {
        "w1p": w1p, "w2p": w2p,
        "b1p": nn1_b.astype(BF), "b2p": b2p.astype(BF),
        "r1wb": r1wb.astype(BF), "r2wb": r2wb.astype(BF),
        "b2sbb": bias2.astype(BF),
        "l1wb": l1wb.astype(BF), "l1brow": (l1b / NCORES).astype(BF),
        "l2wrep": np.tile(l2w, (GSH, 1)).astype(np.float32),
        "l2brep": np.tile(l2b, (GSH, 1)).astype(np.float32),
        "identb": np.eye(P, dtype=BF),
    }

    in_maps = []
    for c in range(NCORES):
        eids = per_core[c]
        ne = len(eids)
        srcs = src[eids]
        dstl = (dst[eids] - c * NSH).astype(np.int64)

        xg = x[srcs, :].astype(BF)  # [ne, 64]
        xsrc2 = np.zeros((P, 2, e_pad), BF)
        for s in range(2):
            iofs = (32 * (g32 + s) + j32) % DN  # [128]
            xsrc2[:, s, 0:ne] = xg[:, iofs].T

        ag = attr[eids, :]  # [ne, 32]
        bcq = np.zeros((P, 8, e_pad), BF)
        for q in range(8):
            for g in range(4):
                bcq[32 * g : 32 * g + 32, q, 0:ne] = ag[:, 4 * q + g].astype(BF)[None, :]

        scm = np.zeros((P, NSC * P), BF)
        for bi, (e, n) in enumerate(sc_blocks):
            seg = dstl[e * P : min((e + 1) * P, ne)]
            for p, dv in enumerate(seg):
                q = dv - n * P
                if 0 <= q < P:
                    scm[p, bi * P + q] = 1.0

        batch_l = batch[c * NSH : (c + 1) * NSH]
        scp = np.zeros((P, NT * GT * P), BF)
        for n in range(NT):
            for g in range(GT):
                blk = n * GT + g
                bseg = batch_l[n * P : (n + 1) * P]
                for p, bv in enumerate(bseg):
                    q = bv - g * P
                    if 0 <= q < P:
                        scp[p, blk * P + q] = BF(recip_g[bv])

        xshT = np.ones((DN + 1, NSH), BF)
        xshT[0:DN, :] = x[c * NSH : (c + 1) * NSH, :].astype(BF).T

        snd_idx = np.full(S, -1, np.int64)
        for d in range(NCORES):
            rows = send_rows[c][d]
            snd_idx[d * SB : d * SB + len(rows)] = rows
        SBT = S // P
        selm = np.zeros((P, SBT * NT * P), BF)
        for row in range(S):
            v = snd_idx[row]
            if v < 0:
                continue
            r, q = row // P, row % P
            nt_, npart = int(v) // P, int(v) % P
            selm[npart, (r * NT + nt_) * P + q] = 1.0
        h1src_idx = np.zeros(e_pad, np.int16)
        for d2 in range(NCORES):
            pos, inv = recv_pos_parts[c][d2]
            h1src_idx[pos] = d2 * SB + inv

        m = dict(common)
        m["xsrc2"] = xsrc2
        m["bcq"] = bcq
        m["scm"] = scm
        m["scp"] = scp
        m["sel"] = selm
        m["xshT"] = xshT
        m["h1src_w"] = _wrap_idx(h1src_idx, e_pad)
        in_maps.append(m)

    zb = (
        bool(np.all(np.asarray(inputs["nn1_b"]) == 0)),
        bool(np.all(np.asarray(inputs["nn2_b"]) == 0))
        and bool(np.all(np.asarray(inputs["bias2"]) == 0)),
        bool(np.all(np.asarray(inputs["lin1_b"]) == 0)),
    )
    _PREP["args"] = (e_pad, S, tuple(sc_blocks), zb)
    return e_pad, in_maps


def kernel(**inputs) -> np.ndarray:
    e_pad, in_maps = _prep_inputs(inputs)
    if e_pad not in _cache:
        ep, S, blocks, zb = _PREP["args"]
        _cache[e_pad] = _build(ep, S, list(blocks), zb=zb)
    nc = _cache[e_pad]
    res = bass_utils.run_bass_kernel_spmd(nc, in_maps, core_ids=list(range(NCORES)))
    return np.asarray(res.results[0]["out"], dtype=np.float32)


def run_debug(upto, **inputs):
    e_pad, in_maps = _prep_inputs(inputs)
    ep, S, blocks, zb = _PREP["args"]
    nc = _build(ep, S, list(blocks), zb=zb, upto=upto)
    res = bass_utils.run_bass_kernel_spmd(nc, in_maps, core_ids=list(range(NCORES)))
    return e_pad, res
